# revision 1
# baseline (speedup 1.0000x reference)
"""RBF causal attention (unnormalized, no softmax denominator) on 8 Trainium2 NeuronCores.

Problem: B=2 H=16 N=2048 D=128 fp32.
  P[m,n] = exp(-s*||q_m - k_n||^2) for m >= n else 0;  O = P @ V
        = exp(2s*(q.k) - s*|q|^2 - s*|k|^2) masked causally.

Sharding: (b*h) = 32 independent slices -> 4 per core across 8 cores, no comms.

Per (b,h) slice on-core algorithm (all layouts chosen so matmuls contract over
the partition dim):
  - load Q,K n-major [128, 16, 128]; V n-major as float32r (matmul fast mode)
  - PE-transpose Q,K 128x128 blocks (fp32, exact) -> QT,KT [d=128, n] staged in
    PSUM, drained to SBUF as float32r (the rounding the f32r matmul requires)
  - q_sq/k_sq: GPSIMD elementwise square + DVE 3-D reduce -> [128, 16]
  - two m-passes (m in [0,1024), [1024,2048)) so PSUM fits:
      for bn in 0..(pass A: 7 / pass B: 15):
        ST[n, m] = KT_bn.T @ QT chunk  (f32r matmul, fp32 PSUM)
        PT = exp(2s*ST - s*k_sq[n])    (ACT, per-partition bias, f32r out)
        diag block: PT *= upper-tri mask (DVE)
        OT[d, m] += V_bn.T @ PT        (f32r matmul, V stationary, fp32 PSUM)
      when a 512-chunk of OT completes: drain -> SBUF, PE-transpose back to
      [m, d], scale by exp(-s*q_sq[m]) per-partition (DVE), assemble, DMA out.
"""

import os
import sys

import numpy as np

_TRN_REPO = "/opt/trn_rl_repo"
if os.path.isdir(_TRN_REPO) and _TRN_REPO not in sys.path:
    sys.path.insert(0, _TRN_REPO)

import concourse.bass as bass  # noqa: E402
import concourse.mybir as mybir  # noqa: E402
import concourse.tile as tile  # noqa: E402
from concourse import bacc  # noqa: E402
from concourse.bass_utils import run_bass_kernel_spmd  # noqa: E402
from concourse.masks import make_identity, make_upper_triangular  # noqa: E402

B, H, N, D = 2, 16, 2048, 128
SM_SCALE = 0.08838834764831845  # 1/sqrt(D)
NCORES = 8
SLICES = (B * H) // NCORES  # per core
NT = N // 128  # 16 row-blocks per slice

F32 = mybir.dt.float32
F32R = mybir.dt.float32r

_nc_cache = None


def _build_nc():
    nc = bacc.Bacc("TRN2", target_bir_lowering=False, debug=False, num_devices=NCORES)

    q_dram = nc.dram_tensor("q", [SLICES, N, D], F32R, kind="ExternalInput").ap()
    k_dram = nc.dram_tensor("k", [SLICES, N, D], F32R, kind="ExternalInput").ap()
    v_dram = nc.dram_tensor("v", [SLICES, N, D], F32R, kind="ExternalInput").ap()
    o_dram = nc.dram_tensor("o", [SLICES, N, D], F32, kind="ExternalOutput").ap()

    with tile.TileContext(nc) as tc:
        singles = tc.alloc_tile_pool(name="singles", bufs=1)
        io = tc.alloc_tile_pool(name="io", bufs=3)
        tqk = tc.alloc_tile_pool(name="tqk", bufs=2)
        ptp = tc.alloc_tile_pool(name="ptp", bufs=3)
        sqp = tc.alloc_tile_pool(name="sqp", bufs=2)
        smalls = tc.alloc_tile_pool(name="smalls", bufs=2)
        osb = tc.alloc_tile_pool(name="osb", bufs=2)
        outp = tc.alloc_tile_pool(name="outp", bufs=2)
        stp = tc.alloc_tile_pool(name="stp", bufs=3, space="PSUM")
        otp = tc.alloc_tile_pool(name="otp", bufs=1, space="PSUM")

        ident = singles.tile([128, 128], F32)
        make_identity(nc, ident)
        identr = singles.tile([128, 128], F32R)
        nc.vector.tensor_copy(identr, ident)
        # trimask[n, m] = 1.0 where m >= n else 0.0 (keep causal upper-incl in [n, m] layout)
        trimask = singles.tile([128, 128], F32)
        make_upper_triangular(nc, trimask, val=1.0, diag=True)

        def emit_in_dma(s):
            kn = io.tile([128, NT, 128], F32R, name=f"kn{s}", tag="kn")
            qn = io.tile([128, NT, 128], F32R, name=f"qn{s}", tag="qn")
            vn = io.tile([128, NT, 128], F32R, name=f"vn{s}", tag="vn")
            nc.sync.dma_start(out=kn, in_=k_dram[s].rearrange("(t p) d -> p t d", p=128))
            nc.sync.dma_start(out=qn, in_=q_dram[s].rearrange("(t p) d -> p t d", p=128))
            nc.sync.dma_start(out=vn, in_=v_dram[s].rearrange("(t p) d -> p t d", p=128))
            return kn, qn, vn

        in_tiles = {0: emit_in_dma(0)}
        for s in range(SLICES):
            kn, qn, vn = in_tiles.pop(s)
            if s + 1 < SLICES:
                # prefetch next slice's inputs so the SP queue works ahead
                in_tiles[s + 1] = emit_in_dma(s + 1)

            # squares: GPSIMD elementwise mul (on fp32 bit-view), DVE reduce over innermost 128
            sk = sqp.tile([128, NT, 128], F32, name=f"sq_k{s}", tag="sq")
            nc.gpsimd.tensor_mul(sk, kn.bitcast(F32), kn.bitcast(F32))
            ksq = smalls.tile([128, NT], F32, name=f"ksq{s}", tag="ksq")
            for r in range(4):
                nc.vector.tensor_reduce(
                    ksq[:, 4 * r : 4 * (r + 1)], sk[:, 4 * r : 4 * (r + 1), :],
                    axis=mybir.AxisListType.X, op=mybir.AluOpType.add)
            ksqb = smalls.tile([128, NT], F32, name=f"ksqb{s}", tag="ksqb")
            nc.gpsimd.tensor_scalar_mul(ksqb, ksq, -SM_SCALE)
            sq = sqp.tile([128, NT, 128], F32, name=f"sq_q{s}", tag="sq")
            nc.gpsimd.tensor_mul(sq, qn.bitcast(F32), qn.bitcast(F32))
            qsq = smalls.tile([128, NT], F32, name=f"qsq{s}", tag="qsq")
            for r in range(4):
                nc.vector.tensor_reduce(
                    qsq[:, 4 * r : 4 * (r + 1)], sq[:, 4 * r : 4 * (r + 1), :],
                    axis=mybir.AxisListType.X, op=mybir.AluOpType.add)
            # eq[m-block] = exp(-s*q_sq)
            eq = smalls.tile([128, NT], F32, name=f"eq{s}", tag="eq")
            nc.scalar.activation(eq, qsq, mybir.ActivationFunctionType.Exp, scale=-SM_SCALE)

            # PE transposes (f32r, 1.5 cyc/row) -> QT/KT [d, n] as f32r
            kt = tqk.tile([128, N], F32R, name=f"kt{s}", tag="kt")
            qt = tqk.tile([128, N], F32R, name=f"qt{s}", tag="qt")
            for src, dst in ((kn, kt), (qn, qt)):
                for h in range(2):
                    stg = stp.tile([128, 1024], F32R, name=f"tstg{s}_{h}", tag="st")
                    for j in range(8):
                        t = 8 * h + j
                        nc.tensor.transpose(stg[:, 128 * j : 128 * (j + 1)], src[:, t, :], identr)
                    nc.vector.tensor_copy(dst[:, 1024 * h : 1024 * (h + 1)], stg)

            o_out = outp.tile([128, NT, 128], F32, name=f"oout{s}", tag="oout")

            for p in range(2):  # m-pass
                mlo = 1024 * p
                ot = otp.tile([128, 1024], F32, name=f"ot{s}_{p}", tag="ot")
                bn_hi = 8 if p == 0 else 16

                def qk_exp(bn, p=p, mlo=mlo, s=s, kt=kt, qt=qt, ksqb=ksqb):
                    """ST = KT_bn.T @ QT chunk; PT = exp(2s*ST - s*k_sq); diag mask."""
                    m0 = max(128 * bn, mlo)
                    w = mlo + 1024 - m0
                    ptt = ptp.tile([128, 1024], F32R, name=f"pt{s}_{p}_{bn}", tag="pt")
                    stt = stp.tile([128, w], F32, name=f"st{s}_{p}_{bn}", tag="st")
                    off = 0
                    while off < w:
                        sw = min(512, w - off)
                        nc.tensor.matmul(
                            stt[:, off : off + sw],
                            kt[:, 128 * bn : 128 * (bn + 1)],
                            qt[:, m0 + off : m0 + off + sw],
                            start=True,
                            stop=True,
                        )
                        off += sw
                    nc.scalar.activation(
                        ptt[:, m0 - mlo : m0 - mlo + w],
                        stt,
                        mybir.ActivationFunctionType.Exp,
                        bias=ksqb[:, bn : bn + 1],
                        scale=2.0 * SM_SCALE,
                    )
                    if 128 * bn >= mlo:
                        nc.vector.tensor_mul(
                            ptt[:, m0 - mlo : m0 - mlo + 128],
                            ptt[:, m0 - mlo : m0 - mlo + 128],
                            trimask,
                        )
                    return ptt

                pts = {0: qk_exp(0)}
                if bn_hi > 1:
                    pts[1] = qk_exp(1)
                for bn in range(bn_hi):
                    # pipeline 2 deep: emit QK/exp for bn+2 before PV(bn) so PE
                    # and ACT always have independent work in the causal tail
                    if bn + 2 < bn_hi:
                        pts[bn + 2] = qk_exp(bn + 2)
                    ptt = pts.pop(bn)
                    m0 = max(128 * bn, mlo)
                    for lc in range(2):
                        cs_abs = mlo + 512 * lc
                        ce_abs = cs_abs + 512
                        ps = max(cs_abs, m0)
                        if ps >= ce_abs:
                            continue
                        c_abs = 2 * p + lc
                        last_bn = 4 * c_abs + 3
                        nc.tensor.matmul(
                            ot[:, ps - mlo : ce_abs - mlo],
                            vn[:, bn, :],
                            ptt[:, ps - mlo : ce_abs - mlo],
                            start=(bn == 0),
                            stop=(bn == last_bn),
                        )
                        if bn == last_bn:
                            # chunk c_abs complete: drain, transpose back, scale
                            otst = osb.tile([128, 512], F32, name=f"otst{s}_{c_abs}", tag="otst")
                            nc.vector.tensor_copy(otst, ot[:, 512 * lc : 512 * (lc + 1)])
                            stg2 = stp.tile([128, 512], F32, name=f"ostg{s}_{c_abs}", tag="st")
                            for j in range(4):
                                nc.tensor.transpose(
                                    stg2[:, 128 * j : 128 * (j + 1)],
                                    otst[:, 128 * j : 128 * (j + 1)],
                                    ident,
                                )
                            for j in range(4):
                                bm = 4 * c_abs + j
                                nc.vector.tensor_scalar_mul(
                                    o_out[:, bm, :],
                                    stg2[:, 128 * j : 128 * (j + 1)],
                                    eq[:, bm : bm + 1],
                                )

            nc.sync.dma_start(out=o_dram[s].rearrange("(t p) d -> p t d", p=128), in_=o_out)

        for pool in (otp, stp, outp, osb, smalls, sqp, ptp, tqk, io, singles):
            pool.release()

    nc.compile()
    return nc


def _get_nc():
    global _nc_cache
    if _nc_cache is None:
        _nc_cache = _build_nc()
    return _nc_cache


def run(q, k, v, trace=False):
    q = np.ascontiguousarray(np.asarray(q, dtype=np.float32))
    k = np.ascontiguousarray(np.asarray(k, dtype=np.float32))
    v = np.ascontiguousarray(np.asarray(v, dtype=np.float32))
    qf = q.reshape(B * H, N, D)
    kf = k.reshape(B * H, N, D)
    vf = v.reshape(B * H, N, D)
    nc = _get_nc()
    in_maps = [
        {
            "q": np.ascontiguousarray(qf[SLICES * i : SLICES * (i + 1)]),
            "k": np.ascontiguousarray(kf[SLICES * i : SLICES * (i + 1)]),
            "v": np.ascontiguousarray(vf[SLICES * i : SLICES * (i + 1)]),
        }
        for i in range(NCORES)
    ]
    res = run_bass_kernel_spmd(nc, in_maps, core_ids=list(range(NCORES)), trace=trace)
    out = np.concatenate([res.results[i]["o"] for i in range(NCORES)], axis=0)
    return out.reshape(B, H, N, D).astype(np.float32), res


def kernel(q, k, v):
    return run(q, k, v)[0]

